# revision 1
# baseline (speedup 1.0000x reference)
"""Trainium2 distributed kernel for nn_AttentionLayer (dense cross-attention
with sink-competition softmax) — v2.

Sharding: 8 cores = 2 batches x 4 head-pairs.  Core c handles batch c//4 and
heads {2*(c%4), 2*(c%4)+1}.

v2 structural changes vs v1:
  - Host ships activations pre-transposed ([feat, tokens], bf16, token-tile
    blocked) so the kernel does zero PE transposes of activations and half
    the HBM traffic.
  - LayerNorm folded into the projections:  LN(x)@W = rstd*(x@W' - c X mu) + b
    with c = gamma@W.  Sums/sums-of-squares come from ones-matmuls against
    the transposed activations (sliding-ones lhsT accumulates all token
    tiles into one [10, 512] PSUM block); squares computed on the otherwise
    idle GpSimd engine; rstd = exp(-0.5*ln(var+eps)) keeps ScalarE in the
    ln/exp table sets only.
  - Key-side rstd is folded into the Exp's per-partition scale AP, query-side
    rstd applied via a rank-1 broadcast, so normalized q/k are never
    materialized separately.
  - sim matmuls for the two heads are row-packed (K=64 at array rows 0-63 /
    64-127) so they run concurrently in the PE array.
  - Output collective is an AllGather of the per-core [128, 1024] bf16
    attention output (instead of ReduceScatter of [1024, 512] f32); each
    core then computes its 256 output rows against the full Wo.
"""

import os
import sys

for _p in ("/opt/trn_rl_repo", "/root/.axon_site/_ro/trn_rl_repo"):
    if os.path.isdir(_p) and _p not in sys.path:
        sys.path.insert(0, _p)

import numpy as np
import ml_dtypes

# Defensive: concourse.bass_utils imports antenv.axon_hooks on the traced
# path; provide a no-op registry if the environment lacks it so tracing
# degrades instead of crashing.
try:
    import antenv.axon_hooks  # noqa: F401
except Exception:
    import types as _types

    _ah = _types.ModuleType("antenv.axon_hooks")
    _ah._hook = None
    _ah.set_axon_ntff_profile_hook = lambda h: setattr(_ah, "_hook", h)
    _ah.get_axon_ntff_profile_hook = lambda: getattr(_ah, "_hook", None)
    try:
        import antenv as _antenv
        _antenv.axon_hooks = _ah
    except Exception:
        pass
    sys.modules["antenv.axon_hooks"] = _ah

import concourse.bass as bass
import concourse.bacc as bacc
import concourse.mybir as mybir
import concourse.tile as tile
from concourse.bass_utils import run_bass_kernel_spmd

F32 = mybir.dt.float32
BF16 = mybir.dt.bfloat16
AF = mybir.ActivationFunctionType
ALU = mybir.AluOpType

B, N_SINK, N_SRC, DIM, HID, H = 2, 1024, 4096, 512, 512, 8
D_HEAD = HID // H            # 64
EPS = 1e-6
SCALE = D_HEAD ** -0.5       # 0.125
N_CORES = 8
GROUP = 4                    # cores per batch group
N_TT = 10                    # token tiles of 512 (2 sink + 8 source)
N_KT = 32                    # key tiles of 128
OUT_ROWS = N_SINK // GROUP   # 256 output rows per core

LAST_RESULT = None


def _stat_row(jt):
    """stats-block row for source key-tile jt (tokens 128*jt..+128)."""
    return 2 + jt // 4


def _col_idx(jt):
    """column in the rstd-cols tiles for key-tile jt."""
    return 10 * (jt % 4) + _stat_row(jt)


def build_bass(has_bias=True):
    nc = bacc.Bacc(None, target_bir_lowering=False, debug=False,
                   num_devices=N_CORES)

    for val in (EPS, SCALE, -0.5, 0.5):
        t = nc.alloc_sbuf_tensor(f"const-f32-{val}", [128, 1], F32)
        nc.gpsimd.memset(t.ap(), val)
        nc.const_aps.aps[(F32, val)] = t.ap()
    nc.all_engine_barrier()

    # ---- per-core DRAM parameters ----
    sinkT_d = nc.declare_dram_parameter("sinkT", [128, 2 * 2048], BF16, isOutput=False)
    srcT_d = nc.declare_dram_parameter("srcT", [128, 8 * 2048], BF16, isOutput=False)
    wq_d = nc.declare_dram_parameter("wq", [128, 512], BF16, isOutput=False)
    wk_d = nc.declare_dram_parameter("wk", [128, 512], BF16, isOutput=False)
    wv_d = nc.declare_dram_parameter("wv", [128, 512], BF16, isOutput=False)
    wo_d = nc.declare_dram_parameter("wo", [128, 4096], BF16, isOutput=False)
    r1q_d = nc.declare_dram_parameter("r1q", [1, 128], BF16, isOutput=False)
    r1k_d = nc.declare_dram_parameter("r1k", [1, 128], BF16, isOutput=False)
    r1v_d = nc.declare_dram_parameter("r1v", [1, 128], BF16, isOutput=False)
    bo4_d = nc.declare_dram_parameter("bo4", [1, 512], BF16, isOutput=False)
    ident_d = nc.declare_dram_parameter("ident", [128, 128], BF16, isOutput=False)
    slide_d = nc.declare_dram_parameter("slide", [128, 32], BF16, isOutput=False)
    onec_d = nc.declare_dram_parameter("ones_c", [128, 1], BF16, isOutput=False)
    oner_d = nc.declare_dram_parameter("ones_r", [1, 512], BF16, isOutput=False)
    bo_d = nc.declare_dram_parameter("bo_r", [1, 512], BF16, isOutput=False)
    out_d = nc.declare_dram_parameter("out", [OUT_ROWS, DIM], F32, isOutput=True)

    # collective bounce buffers: 8-rank AllToAll of attn-out query shards.
    # Shard slot r (rows 128r) of the input = our [128 hid, 256 q] block for
    # rank r's query range (r % 4); after the exchange, output rows 128c hold
    # core c's hid-slice for OUR queries (other-batch rows are masked out of
    # wo on the host).
    a2a_in = nc.dram_tensor("a2a_in", [1024, 256], BF16)
    a2a_out = nc.dram_tensor("a2a_out", [1024, 256], BF16)
    # stats block -> flat-row bounce buffers ([10, 512] -> [1, 5120])
    flat_names = ("ir", "rstd") if has_bias else ("rstd",)
    flat_d = {nm: nc.dram_tensor(f"{nm}_flat_d", [1, 10 * 512], BF16)
              for nm in flat_names}

    with tile.TileContext(nc) as tc:
        with tc.tile_pool(name="const", bufs=1) as cp:
            wq = cp.tile([128, 512], BF16, name="wq_sb")
            wk = cp.tile([128, 512], BF16, name="wk_sb")
            wv = cp.tile([128, 512], BF16, name="wv_sb")
            wo = cp.tile([128, 4096], BF16, name="wo_sb")
            r1q = cp.tile([1, 128], BF16, name="r1q_sb")
            r1k = cp.tile([1, 128], BF16, name="r1k_sb")
            r1v = cp.tile([1, 128], BF16, name="r1v_sb")
            bo4 = cp.tile([1, 512], BF16, name="bo4_sb")
            ident = cp.tile([128, 128], BF16, name="id_sb")
            slide = cp.tile([128, 32], BF16, name="slide_sb")
            ones_c = cp.tile([128, 1], BF16, name="ones_c_sb")
            ones_r = cp.tile([1, 512], BF16, name="ones_r_sb")
            bo_r = cp.tile([1, 512], BF16, name="bo_r_sb")
            # activations, blocked [128, 2048*t + 512*cc + tau]; issue the
            # tensors the stats pipeline needs first.
            xT = cp.tile([128, 20 * 1024], BF16, name="xT_sb")
            nc.sync.dma_start(out=slide[:, :], in_=slide_d[:, :])
            nc.sync.dma_start(out=xT[:, 0:4096], in_=sinkT_d[:, :])
            for piece in range(4):
                # alternate the two HWDGE rings so transfers overlap
                eng = nc.sync if piece % 2 == 0 else nc.scalar
                eng.dma_start(
                    out=xT[:, 4096 * (piece + 1):4096 * (piece + 2)],
                    in_=srcT_d[:, 4096 * piece:4096 * (piece + 1)])
            for sb, dr in ((wq, wq_d), (wk, wk_d), (wv, wv_d), (wo, wo_d),
                           (r1q, r1q_d), (r1k, r1k_d), (r1v, r1v_d),
                           (ident, ident_d), (bo4, bo4_d),
                           (ones_c, onec_d), (ones_r, oner_d), (bo_r, bo_d)):
                nc.sync.dma_start(out=sb[:, :], in_=dr[:, :])

            # persistent activations / stats
            qT = cp.tile([128, 1024], BF16, name="qT_sb")
            kT = cp.tile([128, 4096], BF16, name="kT_sb")
            v_sb = cp.tile([128, 130 * N_KT], BF16, name="v_sb")
            ir_bf = cp.tile([10, 512], BF16, name="ir_bf")
            rstd_bf = cp.tile([10, 512], BF16, name="rstd_bf")
            # flat-row copies (base partition 0) for matmul rank-1 operands
            ir_fl = cp.tile([1, 5120], BF16, name="ir_flat")
            rstd_fl = cp.tile([1, 5120], BF16, name="rstd_flat")
            scol = cp.tile([128, 40], F32, name="scol_sb")
            rcol = cp.tile([128, 40], F32, name="rcol_sb")
            aoT = cp.tile([128, 1024], BF16, name="aoT_sb")
            ao_s = cp.tile([128, 2048], BF16, name="ao_s_sb")

            def xslc(t, cc, off, width):
                return xT[:, 2048 * t + 512 * cc + off:
                          2048 * t + 512 * cc + off + width]

            # ---------------- stats ----------------
            with tc.tile_pool(name="xsq", bufs=6) as sqp, \
                 tc.tile_pool(name="st_ps", bufs=2, space="PSUM") as stp, \
                 tc.tile_pool(name="st_sb", bufs=8) as ssb:
                sx_p = stp.tile([10, 512], F32, tag="st", name="sx_p")
                sx2_p = stp.tile([10, 512], F32, tag="st", name="sx2_p")
                # squares first (split DVE-major/GpSimd), then the plain sums
                # (independent of the squares) so the PE queue never stalls
                # behind a square, then the sum-of-square matmuls.
                sqs = {}
                for t in range(N_TT):
                    for cc in range(4):
                        xs = xslc(t, cc, 0, 512)
                        sq = sqp.tile([128, 512], BF16, tag="sq", bufs=40,
                                      name=f"sq{t}_{cc}")
                        eng = nc.vector if (4 * t + cc) % 5 < 3 else nc.gpsimd
                        eng.tensor_tensor(sq[:, :], xs, xs, ALU.mult)
                        sqs[(t, cc)] = sq
                for t in range(N_TT):
                    lh = slide[:, 10 - t:20 - t]
                    for cc in range(4):
                        nc.tensor.matmul(sx_p[0:10, :], lh,
                                         xslc(t, cc, 0, 512),
                                         start=(t == 0 and cc == 0),
                                         stop=(t == N_TT - 1 and cc == 3))
                for t in range(N_TT):
                    lh = slide[:, 10 - t:20 - t]
                    for cc in range(4):
                        nc.tensor.matmul(sx2_p[0:10, :], lh,
                                         sqs[(t, cc)][:, :],
                                         start=(t == 0 and cc == 0),
                                         stop=(t == N_TT - 1 and cc == 3))

                mu = ssb.tile([10, 512], F32, name="mu_f")
                var = ssb.tile([10, 512], F32, name="var_f")
                lnv = ssb.tile([10, 512], F32, name="lnv_f")
                mu2 = ssb.tile([10, 512], F32, name="mu2_f")
                nc.vector.tensor_scalar(mu[:, :], sx_p[:, :], 1.0 / DIM, None,
                                        ALU.mult)
                nc.vector.tensor_scalar(var[:, :], sx2_p[:, :], 1.0 / DIM,
                                        None, ALU.mult)
                nc.vector.tensor_tensor(mu2[:, :], mu[:, :], mu[:, :],
                                        ALU.mult)
                nc.vector.tensor_tensor(var[:, :], var[:, :], mu2[:, :],
                                        ALU.subtract)
                nc.scalar.activation(lnv[:, :], var[:, :], AF.Ln, bias=EPS)
                nc.scalar.activation(rstd_bf[:, :], lnv[:, :], AF.Exp,
                                     scale=-0.5)
                blks = [(rstd_bf, rstd_fl, "rstd")]
                if has_bias:
                    nc.scalar.activation(ir_bf[:, :], lnv[:, :], AF.Exp,
                                         scale=0.5)
                    blks.append((ir_bf, ir_fl, "ir"))
                # bounce [10, 512] blocks through DRAM into [1, 5120] rows
                for blk_sb, fl_sb, nm in blks:
                    dview = flat_d[nm].ap().rearrange(
                        "a (t n) -> (a t) n", t=10)
                    nc.sync.dma_start(out=dview, in_=blk_sb[:, :])
                    nc.sync.dma_start(out=fl_sb[:, :], in_=flat_d[nm].ap())

                # rstd columns for the source tiles (exp scale + v scaling)
                with tc.tile_pool(name="tp_ps", bufs=2, space="PSUM") as tpp:
                    for c4 in range(4):
                        tp = tpp.tile([128, 10], BF16, tag="tp", name=f"tp{c4}")
                        nc.tensor.transpose(
                            tp[:, :], rstd_bf[0:10, 128 * c4:128 * (c4 + 1)],
                            ident[0:10, 0:10])
                        nc.vector.tensor_scalar(
                            scol[:, 10 * c4:10 * (c4 + 1)], tp[:, :], SCALE,
                            None, ALU.mult)
                        nc.vector.tensor_copy(
                            rcol[:, 10 * c4:10 * (c4 + 1)], tp[:, :])

            # ---------------- projections ----------------
            nc.gpsimd.memset(v_sb[:, :], 1.0)
            with tc.tile_pool(name="pj_ps", bufs=4, space="PSUM") as pjp, \
                 tc.tile_pool(name="bc_ps", bufs=1, space="PSUM") as bcp, \
                 tc.tile_pool(name="bc_sb", bufs=2) as bsb, \
                 tc.tile_pool(name="vp_ps", bufs=3, space="PSUM") as vpp:
                # q projection (2 token tiles)
                for g in range(2):
                    pj = pjp.tile([128, 512], F32, tag="pj", name=f"pjq{g}")
                    for cc in range(4):
                        nc.tensor.matmul(pj[:, :],
                                         wq[:, 128 * cc:128 * (cc + 1)],
                                         xslc(g, cc, 0, 512),
                                         start=(cc == 0),
                                         stop=(cc == 3 and not has_bias))
                    if has_bias:
                        nc.tensor.matmul(pj[:, :], r1q[0:1, :],
                                         ir_fl[0:1, 512 * g:512 * (g + 1)],
                                         start=False, stop=True)
                    bcq = bcp.tile([128, 512], F32, tag="bc", name=f"bcq{g}")
                    nc.tensor.matmul(bcq[:, :], ones_r[0:1, 0:128],
                                     rstd_fl[0:1, 512 * g:512 * (g + 1)],
                                     start=True, stop=True)
                    bcs = bsb.tile([128, 512], BF16, tag="bcs", name=f"bcs{g}")
                    nc.vector.tensor_copy(bcs[:, :], bcq[:, :])
                    nc.vector.tensor_tensor(qT[:, 512 * g:512 * (g + 1)],
                                            pj[:, :], bcs[:, :], ALU.mult)

                # k^T projection (8 token tiles)
                for g in range(8):
                    t = 2 + g
                    pj = pjp.tile([128, 512], F32, tag="pj", name=f"pjk{g}")
                    for cc in range(4):
                        nc.tensor.matmul(pj[:, :],
                                         wk[:, 128 * cc:128 * (cc + 1)],
                                         xslc(t, cc, 0, 512),
                                         start=(cc == 0),
                                         stop=(cc == 3 and not has_bias))
                    if has_bias:
                        nc.tensor.matmul(pj[:, :], r1k[0:1, :],
                                         ir_fl[0:1, 512 * t:512 * (t + 1)],
                                         start=False, stop=True)
                    nc.scalar.activation(kT[:, 512 * g:512 * (g + 1)],
                                         pj[:, :], AF.Copy)

                # v projection (32 key tiles), natural [keys, dims] layout
                for jt in range(N_KT):
                    t, w = _stat_row(jt), 128 * (jt % 4)
                    vp = vpp.tile([128, 128], F32, tag="vp", name=f"vp{jt}")
                    for cc in range(4):
                        nc.tensor.matmul(vp[:, :], xslc(t, cc, w, 128),
                                         wv[:, 128 * cc:128 * (cc + 1)],
                                         start=(cc == 0),
                                         stop=(cc == 3 and not has_bias))
                    if has_bias:
                        nc.tensor.matmul(
                            vp[:, :],
                            ir_fl[0:1, 512 * t + w:512 * t + w + 128],
                            r1v[0:1, :], start=False, stop=True)
                    vb = v_sb[:, 130 * jt:130 * jt + 130]
                    nc.vector.tensor_scalar(vb[:, 0:64], vp[:, 0:64],
                                            rcol[:, _col_idx(jt):_col_idx(jt) + 1],
                                            None, ALU.mult)
                    nc.vector.tensor_scalar(vb[:, 65:129], vp[:, 64:128],
                                            rcol[:, _col_idx(jt):_col_idx(jt) + 1],
                                            None, ALU.mult)

            # ---------------- attention ----------------
            with tc.tile_pool(name="acc_ps", bufs=2, space="PSUM") as accp:
                acc = [accp.tile([65, 1024], F32, tag="acc", name=f"acc{h}")
                       for h in range(2)]
                with tc.tile_pool(name="sim_ps", bufs=2, space="PSUM") as simp, \
                     tc.tile_pool(name="att", bufs=4) as ap_, \
                     tc.tile_pool(name="rs", bufs=3) as rsp:
                    exs = {}
                    lts = {}

                    def issue_sim(jt):
                        """sim matmuls + exp + lt build for key tile jt."""
                        ci = _col_idx(jt)
                        s2 = rsp.tile([128, 2], F32, tag="s2",
                                      name=f"s2_{jt}")
                        for h in range(2):
                            hs = 64 * h
                            sim = simp.tile([128, 1024], F32, tag="sim",
                                            name=f"sim{jt}_{h}")
                            for qc in range(2):
                                nc.tensor.matmul(
                                    sim[:, 512 * qc:512 * (qc + 1)],
                                    kT[hs:hs + 64, 128 * jt:128 * (jt + 1)],
                                    qT[hs:hs + 64, 512 * qc:512 * (qc + 1)],
                                    start=True, stop=True)
                            ex = ap_.tile([128, 1024], BF16, tag="ex",
                                          bufs=6, name=f"ex{jt}_{h}")
                            nc.scalar.activation(ex[:, :], sim[:, :],
                                                 AF.Exp,
                                                 scale=scol[:, ci:ci + 1],
                                                 accum_out=s2[:, h:h + 1])
                            exs[(jt, h)] = ex
                        rs2 = rsp.tile([128, 2], F32, tag="rs2",
                                       name=f"rs2_{jt}")
                        nc.vector.reciprocal(rs2[:, :], s2[:, :])
                        lt = ap_.tile([128, 130], BF16, tag="lt", bufs=3,
                                      name=f"lt{jt}")
                        vb = v_sb[:, 130 * jt:130 * jt + 130]
                        nc.vector.tensor_scalar(lt[:, 0:65], vb[:, 0:65],
                                                rs2[:, 0:1], None, ALU.mult)
                        nc.vector.tensor_scalar(lt[:, 65:130], vb[:, 65:130],
                                                rs2[:, 1:2], None, ALU.mult)
                        lts[jt] = lt

                    def issue_av(jt):
                        lt = lts.pop(jt)
                        for h in range(2):
                            ex = exs.pop((jt, h))
                            for qc in range(2):
                                nc.tensor.matmul(
                                    acc[h][0:65, 512 * qc:512 * (qc + 1)],
                                    lt[:, 65 * h:65 * (h + 1)],
                                    ex[:, 512 * qc:512 * (qc + 1)],
                                    start=(jt == 0), stop=(jt == N_KT - 1))

                    issue_sim(0)
                    for jt in range(N_KT):
                        if jt + 1 < N_KT:
                            issue_sim(jt + 1)
                        issue_av(jt)

                # normalize by C (1/C = exp(-ln C) on ScalarE) and emit
                # attn_out^T (bf16)
                with tc.tile_pool(name="ep_sb", bufs=2) as epp, \
                     tc.tile_pool(name="ep_ps", bufs=1, space="PSUM") as epps:
                    bc = epps.tile([128, 1024], F32, name="bc_ps")
                    for h in range(2):
                        lnC = epp.tile([1, 1024], F32, tag="lnC",
                                       name=f"lnC{h}")
                        nc.scalar.activation(lnC[:, :], acc[h][64:65, :],
                                             AF.Ln)
                        rcb = epp.tile([1, 1024], BF16, tag="rcb",
                                       name=f"rcb{h}")
                        nc.scalar.activation(rcb[:, :], lnC[:, :], AF.Exp,
                                             scale=-1.0)
                        for qc in range(2):
                            nc.tensor.matmul(
                                bc[64 * h:64 * (h + 1),
                                   512 * qc:512 * (qc + 1)],
                                ones_r[0:1, 0:64],
                                rcb[0:1, 512 * qc:512 * (qc + 1)],
                                start=True, stop=True)
                    bcs2 = epp.tile([128, 1024], BF16, tag="bcs2",
                                    name="bcs2")
                    nc.vector.tensor_copy(bcs2[:, :], bc[:, :])
                    for h in range(2):
                        nc.vector.tensor_tensor(
                            aoT[64 * h:64 * (h + 1), :], acc[h][0:64, :],
                            bcs2[64 * h:64 * (h + 1), :], ALU.mult)

            # ------- 8-rank AllToAll of query shards + final projection -------
            aoT3 = aoT[:, :].rearrange("p (j n) -> p j n", j=4)
            for half in range(2):
                dst = a2a_in.ap()[512 * half:512 * (half + 1), :].rearrange(
                    "(j p) n -> p j n", j=4)
                nc.sync.dma_start(out=dst, in_=aoT3)
            nc.gpsimd.collective_compute(
                "AllToAll", ALU.bypass,
                replica_groups=[[0, 1, 2, 3, 4, 5, 6, 7]],
                ins=[a2a_in.ap().opt()],
                outs=[a2a_out.ap().opt()],
            )
            nc.sync.dma_start(
                out=ao_s[:, :].rearrange("p (r n) -> p r n", r=8),
                in_=a2a_out.ap().rearrange("(r p) n -> p r n", r=8))

            with tc.tile_pool(name="f_ps", bufs=2, space="PSUM") as fpp, \
                 tc.tile_pool(name="fout", bufs=2) as fop:
                for q2 in range(2):
                    f = fpp.tile([128, 512], F32, tag="f", name=f"f{q2}")
                    for blk in range(8):
                        nc.tensor.matmul(
                            f[:, :],
                            ao_s[:, 256 * blk + 128 * q2:
                                 256 * blk + 128 * (q2 + 1)],
                            wo[:, 512 * blk:512 * (blk + 1)],
                            start=(blk == 0),
                            stop=(blk == 7 and not has_bias))
                    if has_bias:
                        nc.tensor.matmul(f[:, :], ones_r[0:1, 0:128],
                                         bo_r[0:1, :], start=False, stop=True)
                    fo = fop.tile([128, 512], F32, tag="fo", name=f"fo{q2}")
                    nc.vector.tensor_copy(fo[:, :], f[:, :])
                    nc.sync.dma_start(out=out_d[128 * q2:128 * (q2 + 1), :],
                                      in_=fo[:, :])

    return nc


def _blk(xT):
    """[512, T] f32 -> [128, 4*T] bf16, col = 2048*t + 512*cc + tau."""
    T = xT.shape[1]
    nt = T // 512
    out = xT.reshape(4, 128, nt, 512).transpose(1, 2, 0, 3).reshape(128, 4 * T)
    return np.ascontiguousarray(out).astype(ml_dtypes.bfloat16)


def _chunked(w_loc):
    """[512, 128] -> [128, 512] with col = 128*cc + d."""
    return np.ascontiguousarray(
        w_loc.reshape(4, 128, 128).transpose(1, 0, 2).reshape(128, 512))


def make_in_maps(sink, source, gamma_s, beta_s, gamma_c, beta_c,
                 Wq, bq, Wkv, bkv, Wo, bo):
    f32 = np.float32
    bf16 = ml_dtypes.bfloat16
    cq = (gamma_s @ Wq).astype(f32)
    ck = (gamma_c @ Wkv[:, :HID]).astype(f32)
    cv = (gamma_c @ Wkv[:, HID:]).astype(f32)
    # LN fold: rstd_i*(x_i @ W_eff) + b_eff == LN(x_i) @ (gamma*W) + b, with
    # the mean correction folded into the weights as a rank-1 update.
    Wq_eff = (Wq * gamma_s[:, None] - cq[None, :] / DIM).astype(f32)
    bq_eff = (bq + beta_s @ Wq).astype(f32)
    Wkv_eff = (Wkv * gamma_c[:, None]
               - np.concatenate([ck, cv])[None, :] / DIM).astype(f32)
    bkv_eff = (bkv + beta_c @ Wkv).astype(f32)
    Wk_f, Wv_f = Wkv_eff[:, :HID], Wkv_eff[:, HID:]
    bk_f, bv_f = bkv_eff[:HID], bkv_eff[HID:]

    ident = np.eye(128, dtype=f32).astype(bf16)
    slide = np.zeros((128, 32), f32)
    slide[:, 10] = 1.0
    slide = slide.astype(bf16)
    ones_c = np.ones((128, 1), f32).astype(bf16)
    ones_r = np.ones((1, 512), f32).astype(bf16)
    bo_r = bo.reshape(1, 512).astype(bf16)
    bo4 = (bo / GROUP).reshape(1, 512).astype(bf16)

    def wo_mask(b):
        """[128, 8*512]: block c = Wo rows for core c's hid slice, zeroed
        when core c belongs to the other batch."""
        blocks = []
        for c in range(N_CORES):
            if c // GROUP == b:
                blocks.append(Wo[128 * (c % GROUP):128 * (c % GROUP + 1), :])
            else:
                blocks.append(np.zeros((128, 512), np.float32))
        return np.concatenate(blocks, axis=1)

    in_maps = []
    for c in range(N_CORES):
        b, hp = c // GROUP, c % GROUP
        cols = slice(128 * hp, 128 * hp + 128)
        in_maps.append({
            "sinkT": _blk(np.ascontiguousarray(sink[b].T).astype(f32)),
            "srcT": _blk(np.ascontiguousarray(source[b].T).astype(f32)),
            "wq": _chunked(Wq_eff[:, cols]).astype(bf16),
            "wk": _chunked(Wk_f[:, cols]).astype(bf16),
            "wv": _chunked(Wv_f[:, cols]).astype(bf16),
            "wo": wo_mask(b).astype(bf16),
            "r1q": bq_eff[cols][None, :].astype(bf16),
            "r1k": bk_f[cols][None, :].astype(bf16),
            "r1v": bv_f[cols][None, :].astype(bf16),
            "bo4": bo4,
            "ident": ident,
            "slide": slide,
            "ones_c": ones_c,
            "ones_r": ones_r,
            "bo_r": bo_r,
        })
    return in_maps


_NC_CACHE = {}


def kernel(**inputs):
    global LAST_RESULT
    has_bias = bool(
        np.any(inputs["bq"]) or np.any(inputs["bkv"]) or np.any(inputs["bo"])
        or np.any(inputs["beta_s"]) or np.any(inputs["beta_c"]))
    if has_bias not in _NC_CACHE:
        nc = build_bass(has_bias)
        if not nc.is_finalized():
            nc.finalize()
        _NC_CACHE[has_bias] = nc
    nc = _NC_CACHE[has_bias]
    in_maps = make_in_maps(**inputs)
    res = run_bass_kernel_spmd(nc, in_maps, core_ids=list(range(N_CORES)))
    LAST_RESULT = res
    outs = res.results
    full = np.empty((B, N_SINK, DIM), np.float32)
    for b in range(B):
        full[b] = np.concatenate(
            [outs[GROUP * b + r]["out"] for r in range(GROUP)], axis=0)
    return full

